# revision 2
# baseline (speedup 1.0000x reference)
"""QKV projection kernel for 8 TRN2 NeuronCores.

Computes qkv = hidden_states @ qkv_proj.T, split into q, k, v heads.
Sharding: data-parallel over tokens (batch*seq = 16384 rows / 8 cores = 2048
rows per core); qkv_proj replicated. Per-core matmul:
  [2048, 4096] @ [4096, 12288] in fp32r (tf32-like, full PE rate, ~1.5e-4 rel).

DRAM layouts are pre-tiled on host so every DMA is contiguous:
  x   [128, 32, 2048]  : x[p, ko, m] = hidden[m_global, ko*128+p]
  w   [128, 32, 12288] : w[p, ko, n] = qkv_proj[n, ko*128+p]
  out [128, 16, 12288] : out[p, mo, n] = qkv[mo*128+p, n]
"""

import sys
import types

import numpy as np

# bass_utils' trace path imports antenv.axon_hooks, which some environments
# lack; provide a no-op fallback so a BASS_TRACE=1 caller doesn't crash.
try:
    import antenv.axon_hooks  # noqa: F401
except ImportError:
    import antenv

    _m = types.ModuleType("antenv.axon_hooks")
    _m._hook = None
    _m.set_axon_ntff_profile_hook = lambda h: setattr(_m, "_hook", h)
    _m.get_axon_ntff_profile_hook = lambda: _m._hook
    sys.modules["antenv.axon_hooks"] = _m
    antenv.axon_hooks = _m

import concourse.bacc as bacc
import concourse.mybir as mybir
import concourse.tile as tile
from concourse._compat import get_trn_type
from concourse.bass_utils import run_bass_kernel_spmd

P = 128
EMBED = 4096
KO = EMBED // P            # 32 k-subtiles
NQKV = 3 * EMBED           # 12288
TOKENS = 16384
N_CORES = 8
M_CORE = TOKENS // N_CORES  # 2048 tokens per core
NT = 512                   # matmul moving free dim (fp32 max, full fp32r rate)
N_TILES = NQKV // NT       # 24
KH = 8                     # k-subtiles per W DMA subtile
KGROUPS = KO // KH         # 4
BLOCKS = [(0, 768), (768, 768), (1536, 512)]   # m-blocks cached in SBUF

f32 = mybir.dt.float32
f32r = mybir.dt.float32r

_CACHE = {}
LAST_RESULTS = None


def _build():
    nc = bacc.Bacc(get_trn_type() or "TRN2", target_bir_lowering=False, debug=False)
    x_d = nc.dram_tensor("x", (P, KO, M_CORE), f32r, kind="ExternalInput")
    w_d = nc.dram_tensor("w", (P, KO, NQKV), f32r, kind="ExternalInput")
    out_d = nc.dram_tensor("out", (P, M_CORE // P, NQKV), f32, kind="ExternalOutput")

    with tile.TileContext(nc) as tc:
        with tc.tile_pool(name="xpool", bufs=1) as xpool, \
             tc.tile_pool(name="wpool", bufs=6) as wpool, \
             tc.tile_pool(name="pspool", bufs=4, space="PSUM") as pspool, \
             tc.tile_pool(name="opool", bufs=4) as opool:
            for (m0, mlen) in BLOCKS:
                x_blk = xpool.tile([P, KO, mlen], f32r, tag="x", name="x_blk")
                for kq in range(4):
                    nc.sync.dma_start(
                        x_blk[:, kq * 8:(kq + 1) * 8, :],
                        x_d[:, kq * 8:(kq + 1) * 8, m0:m0 + mlen],
                    )
                for nt in range(N_TILES):
                    n0 = nt * NT
                    wsubs = []
                    for kh in range(KGROUPS):
                        wt = wpool.tile([P, KH, NT], f32r, tag="w", name="w_sub")
                        nc.sync.dma_start(
                            wt[:], w_d[:, kh * KH:(kh + 1) * KH, n0:n0 + NT]
                        )
                        wsubs.append(wt)
                    for mt in range(mlen // P):
                        ps = pspool.tile([P, NT], f32, tag="ps", name="ps")
                        for kh in range(KGROUPS):
                            for kk in range(KH):
                                ko = kh * KH + kk
                                nc.tensor.matmul(
                                    ps[:],
                                    x_blk[:, ko, mt * P:(mt + 1) * P],
                                    wsubs[kh][:, kk, :],
                                    start=(ko == 0),
                                    stop=(ko == KO - 1),
                                )
                        o_sb = opool.tile([P, NT], f32, tag="o", name="o_sb")
                        nc.vector.tensor_copy(o_sb[:], ps[:])
                        mo = m0 // P + mt
                        nc.sync.dma_start(out_d[:, mo, n0:n0 + NT], o_sb[:])

    nc.compile()
    return nc


def kernel(hidden_states, qkv_proj, position_ids=None, **_unused):
    global LAST_RESULTS
    x = np.ascontiguousarray(hidden_states, dtype=np.float32).reshape(TOKENS, EMBED)
    w = np.ascontiguousarray(qkv_proj, dtype=np.float32)

    if "nc" not in _CACHE:
        _CACHE["nc"] = _build()
    nc = _CACHE["nc"]

    # w tiled: [4096, 12288] -> [128, 32, 12288] with k = ko*128 + p
    w_t = np.ascontiguousarray(
        w.T.reshape(KO, P, NQKV).transpose(1, 0, 2)
    )
    in_maps = []
    for i in range(N_CORES):
        xs = x[i * M_CORE:(i + 1) * M_CORE]              # [2048, 4096]
        x_t = np.ascontiguousarray(
            xs.T.reshape(KO, P, M_CORE).transpose(1, 0, 2)
        )                                                # [128, 32, 2048]
        in_maps.append({"x": x_t, "w": w_t})

    res = run_bass_kernel_spmd(nc, in_maps, core_ids=list(range(N_CORES)))
    LAST_RESULTS = res

    parts = [
        res.results[i]["out"].transpose(1, 0, 2).reshape(M_CORE, NQKV)
        for i in range(N_CORES)
    ]
    qkv = np.concatenate(parts, axis=0)                  # [16384, 12288]
    query = np.ascontiguousarray(qkv[:, :EMBED]).reshape(TOKENS, 32, 128)
    key = np.ascontiguousarray(qkv[:, EMBED:2 * EMBED]).reshape(TOKENS, 32, 128)
    value = np.ascontiguousarray(qkv[:, 2 * EMBED:]).reshape(TOKENS, 32, 128)
    return (query, key, value)
